# revision 36
# baseline (speedup 1.0000x reference)
"""Trainium2 Bass kernel for LocationSensitiveAttention.

Strategy (data-parallel over batch, 8 cores x 16 batches):
  per batch b:
    V   [128, 8, 512]  <- memory[b] natural layout (t on partitions, f32r)
    VT  [128, 4, 1024] <- PE-transposed V (d on partitions) via 32 128x128 transposes
    comb[128(a), 512(t)] psum x2 = W_memory^T @ V^T  +  M2plus^T @ XTplus
        where M2plus = [conv_kernel@W_loc ; pq_b] (pq folded via ones row in XT)
    tanh -> [128(a), 1024(t)] sbuf (ACT)
    energies [128(t), 8(tc)] psum via 8 matmuls with v as rhs
    softmax over 1024 = (partitions x 8 cols), no max-sub (|e| <= ||v||_1 ~ 9)
    context [1, 512] psum = sum_t attn[t] * V[t, :]  (8 accumulating matmuls)
All heavy matmul operands are float32r (fp32 RNE-rounded to 11 mantissa bits,
pre-rounded on host), which streams at 1 cyc/row on the PE (4x faster than fp32).
"""
import sys

sys.path.insert(0, "/opt/trn_rl_repo")
import numpy as np

import concourse.bacc as bacc
import concourse.bass as bass
import concourse.mybir as mybir
import concourse.tile as tile
from concourse.bass_utils import run_bass_kernel_spmd

F32 = mybir.dt.float32
F32R = mybir.dt.float32r
AP = bass.AP

NCORES = 8
B, T, D, Q, A, NF, KW = 128, 1024, 512, 1024, 128, 32, 31
BL = B // NCORES          # 16 batches per core
TC = T // 128             # 8 t-chunks
DC = D // 128             # 4 d-chunks
KC = 2 * KW               # 62 conv taps (c-major: rows 0..30 prev, 31..61 cum)
PAD = (KW - 1) // 2       # 15
TP = T + 2 * PAD          # 1054
NEG = -1.0e38

LAST_EXEC_NS = None


def f32r_round(x):
    """Host equivalent of the device fp32->fp32r cast (RNE to 11 mantissa bits)."""
    u = np.ascontiguousarray(x, dtype=np.float32).view(np.uint32).astype(np.uint64)
    tie = (u >> np.uint64(12)) & np.uint64(1)
    r = (((u + np.uint64(0x7FF) + tie) >> np.uint64(12)) << np.uint64(12)).astype(np.uint32)
    return r.view(np.float32)


def build_program(n_batches=BL):
    nc = bacc.Bacc(None, target_bir_lowering=False)
    d = {}

    def inp(name, shape, dt):
        d[name] = nc.dram_tensor(name, list(shape), dt, kind="ExternalInput")
        return d[name]

    def outp(name, shape, dt):
        d[name] = nc.dram_tensor(name, list(shape), dt, kind="ExternalOutput")
        return d[name]

    mem_d = inp("mem", (n_batches, T, D), F32R)
    ppad_d = inp("ppad", (n_batches, TP), F32R)
    cpad_d = inp("cpad", (n_batches, TP), F32R)
    pcum_d = inp("pcum", (n_batches, T), F32)
    qt_d = inp("qt", (Q, n_batches), F32R)
    wq_d = inp("wq", (Q, A), F32R)
    wm_d = inp("wm", (D, A), F32R)
    m2_d = inp("m2", (KC, n_batches * A), F32R)
    v_d = inp("v", (A, 2), F32R)
    ident_d = inp("ident", (128, 128), F32R)
    onesrow_d = inp("onesrow", (1, T), F32R)
    onesm_d = inp("onesm", (1, 128), F32R)
    # staircase mask decomposition: pen[p,c] = NEG*(u[c] + w[p]*z[c])
    # u/z duplicated x2 to match the duplicated-energy column layout
    masku_d = inp("masku", (1, n_batches * 2 * TC), F32R)
    maskw_d = inp("maskw", (1, n_batches * 128), F32R)
    maskz_d = inp("maskz", (1, n_batches * 2 * TC), F32R)

    ctx_d = outp("ctx_o", (n_batches, D), F32)
    attn_d = outp("attn_o", (n_batches, T), F32)
    ncum_d = outp("ncum_o", (n_batches, T), F32)

    Add = mybir.AluOpType.add
    Tanh = mybir.ActivationFunctionType.Tanh
    Exp = mybir.ActivationFunctionType.Exp

    with tile.TileContext(nc) as tc:
        with (
            tc.tile_pool(name="static", bufs=1) as st,
            tc.tile_pool(name="vpool", bufs=20) as vpool,
            tc.tile_pool(name="vtpool", bufs=2) as vtpool,
            tc.tile_pool(name="xtpool", bufs=2) as xtpool,
            tc.tile_pool(name="thpool", bufs=2) as thpool,
            tc.tile_pool(name="small", bufs=4) as sm,
            tc.tile_pool(name="vtps", bufs=3, space=bass.MemorySpace.PSUM) as vtps_p,
            tc.tile_pool(name="comb", bufs=2, space=bass.MemorySpace.PSUM) as comb_p,
            tc.tile_pool(name="ep", bufs=3, space=bass.MemorySpace.PSUM) as ep_p,
        ):
            # ---- statics ----
            wm = st.tile([128, DC, A], F32R, tag="wm")
            nc.sync.dma_start(wm[:], AP(wm_d, 0, [[A, 128], [128 * A, DC], [1, A]]))
            wq = st.tile([128, TC, A], F32R, tag="wq")
            nc.sync.dma_start(wq[:], AP(wq_d, 0, [[A, 128], [128 * A, TC], [1, A]]))
            qt = st.tile([128, TC, n_batches], F32R, tag="qt")
            nc.sync.dma_start(
                qt[:], AP(qt_d, 0, [[n_batches, 128], [128 * n_batches, TC], [1, n_batches]])
            )
            m2p = st.tile([KC + 1, n_batches * A], F32R, tag="m2p")
            nc.sync.dma_start(m2p[0:KC, :], m2_d[:])
            ident = st.tile([128, 128], F32R, tag="ident")
            nc.sync.dma_start(ident[:], ident_d[:])
            vv = st.tile([A, 2], F32R, tag="vv")
            nc.sync.dma_start(vv[:], v_d[:])
            onesm = st.tile([1, 128], F32R, tag="onesm")
            nc.sync.dma_start(onesm[:], onesm_d[:])
            masku = st.tile([1, n_batches * 2 * TC], F32R, tag="masku")
            nc.sync.dma_start(masku[:], masku_d[:])
            maskw = st.tile([1, n_batches * 128], F32R, tag="maskw")
            nc.sync.dma_start(maskw[:], maskw_d[:])
            maskz = st.tile([1, n_batches * 2 * TC], F32R, tag="maskz")
            nc.sync.dma_start(maskz[:], maskz_d[:])

            # ---- pq = query @ W_query, scattered into row 62 of m2p ----
            pq_ps = ep_p.tile([n_batches, A], F32, tag="ep")
            for c in range(TC):
                nc.tensor.matmul(
                    pq_ps[:], qt[:, c, :], wq[:, c, :], start=(c == 0), stop=(c == TC - 1)
                )
            pq_sb = sm.tile([n_batches, A], F32R, tag="pq_sb")
            nc.vector.tensor_copy(pq_sb[:], pq_ps[:])
            nc.sync.dma_start(m2p[KC : KC + 1, :], pq_sb[:])

            state = {}

            def stage_a(b):
                # ---- load memory[b] natural: [128(t%128), 8(tc), 512(d)] ----
                V = [
                    vpool.tile([128, D], F32R, tag="V", name=f"V_{b}_{i}")
                    for i in range(TC)
                ]
                for tcc in range(TC):
                    nc.sync.dma_start(
                        V[tcc][:],
                        AP(mem_d, (b * T + tcc * 128) * D, [[D, 128], [1, D]]),
                    )
                # ---- im2col windows for the location conv (+ ones row) ----
                XT = xtpool.tile([KC + 1, T], F32R, tag="XT", name=f"XT_{b}")
                nc.sync.dma_start(XT[0:KW, :], AP(ppad_d, b * TP, [[1, KW], [1, T]]))
                nc.sync.dma_start(XT[KW:KC, :], AP(cpad_d, b * TP, [[1, KW], [1, T]]))
                nc.sync.dma_start(XT[KC : KC + 1, :], onesrow_d[:])

                # ---- transpose V -> VT [128(d%128), 4(dc), 1024(t)] ----
                VT = vtpool.tile([128, DC, T], F32R, tag="VT", name=f"VT_{b}")
                for h in range(2):
                    for dc in range(DC):
                        ps = vtps_p.tile([128, 512], F32R, tag="vtps")
                        for i in range(4):
                            tcc = h * 4 + i
                            nc.tensor.matmul(
                                ps[:, i * 128 : (i + 1) * 128],
                                V[tcc][:, dc * 128 : (dc + 1) * 128],
                                ident[:],
                                is_transpose=True,
                                start=(i == 0),
                                stop=(i == 3),
                            )
                        eng = nc.vector if (dc % 2 == 0) else nc.scalar
                        if eng is nc.vector:
                            nc.vector.tensor_copy(
                                VT[:, dc, h * 512 : (h + 1) * 512], ps[:]
                            )
                        else:
                            nc.scalar.copy(VT[:, dc, h * 512 : (h + 1) * 512], ps[:])

                # ---- comb = keys^T + ploc^T + pq  (PSUM [128(a), 512(t)] x2) ----
                th = thpool.tile([128, T], F32R, tag="th", name=f"th_{b}")
                for h in range(2):
                    cps = comb_p.tile([128, 512], F32, tag="comb")
                    for dc in range(DC):
                        nc.tensor.matmul(
                            cps[:],
                            wm[:, dc, :],
                            VT[:, dc, h * 512 : (h + 1) * 512],
                            start=(dc == 0),
                            stop=False,
                        )
                    nc.tensor.matmul(
                        cps[:],
                        m2p[:, b * A : (b + 1) * A],
                        XT[:, h * 512 : (h + 1) * 512],
                        start=False,
                        stop=True,
                    )
                    nc.scalar.activation(th[:, h * 512 : (h + 1) * 512], cps[:], Tanh)

                # ---- energies [128(t), 2*8] (each energy duplicated; f32r N>=2) ----
                eps = ep_p.tile([128, 2 * TC], F32, tag="ep", name=f"eps_{b}")
                for tcc in range(TC):
                    nc.tensor.matmul(
                        eps[:, 2 * tcc : 2 * tcc + 2],
                        th[:, tcc * 128 : (tcc + 1) * 128],
                        vv[:],
                        start=(tcc == 0),
                        stop=False,
                    )
                # pen[p,c] = NEG*(u[c] + w[p]*z[c]) as two rank-1 accumulations
                nc.tensor.matmul(
                    eps[:], onesm[:], masku[:, b * 2 * TC : (b + 1) * 2 * TC],
                    start=False, stop=False,
                )
                nc.tensor.matmul(
                    eps[:],
                    maskw[:, b * 128 : (b + 1) * 128],
                    maskz[:, b * 2 * TC : (b + 1) * 2 * TC],
                    start=False, stop=True,
                )

                # ---- softmax over (partition,col), no max-sub needed ----
                # exp in f32r feeds context directly; normalization deferred
                expm = sm.tile([128, TC], F32R, tag="expm", name=f"expm_{b}")
                scol = sm.tile([128, 1], F32, tag="scol", name=f"scol_{b}")
                nc.scalar.activation(
                    expm[:], eps[:, 0 : 2 * TC : 2], Exp, accum_out=scol[:]
                )
                state[b] = (V, expm, scol)

            def stage_b(b):
                V, expm, scol = state.pop(b)
                # ---- context (unnormalized) [1, 512] -- keeps PE path short ----
                cxp = ep_p.tile([1, D], F32, tag="ep", name=f"cxp_{b}")
                for tcc in range(TC):
                    nc.tensor.matmul(
                        cxp[:],
                        expm[:, tcc : tcc + 1],
                        V[tcc][:],
                        start=(tcc == 0),
                        stop=(tcc == TC - 1),
                    )
                # ---- normalization path (off the PE critical path) ----
                sall = sm.tile([128, 1], F32, tag="sall", name=f"sall_{b}")
                nc.gpsimd.partition_all_reduce(
                    sall[:], scol[:], channels=128, reduce_op=bass.bass_isa.ReduceOp.add
                )
                r128 = sm.tile([128, 1], F32, tag="r128", name=f"r128_{b}")
                nc.vector.reciprocal(r128[:], sall[:])
                attn = sm.tile([128, TC], F32, tag="attn", name=f"attn_{b}")
                nc.vector.tensor_scalar_mul(attn[:], expm[:].bitcast(F32), r128[:])
                cxr = sm.tile([1, D], F32, tag="cxr", name=f"cxr_{b}")
                nc.scalar.mul(cxr[:], cxp[:], r128[0:1, :])
                nc.gpsimd.dma_start(ctx_d[b : b + 1, :], cxr[:])

                # ---- transpose attn [128,8] -> rows [8,128] for efficient IO ----
                atp = ep_p.tile([TC, 128], F32, tag="ep", name=f"atp_{b}")
                nc.tensor.matmul(
                    atp[:], attn[:], ident[:].bitcast(F32),
                    is_transpose=True, start=True, stop=True,
                )
                attn_row = sm.tile([TC, 128], F32, tag="attn_row", name=f"attn_row_{b}")
                nc.scalar.copy(attn_row[:], atp[:])
                nc.gpsimd.dma_start(AP(attn_d, b * T, [[128, TC], [1, 128]]), attn_row[:])
                # ---- cumulative weights in row layout ----
                pcr = sm.tile([TC, 128], F32, tag="pcr", name=f"pcr_{b}")
                nc.gpsimd.dma_start(pcr[:], AP(pcum_d, b * T, [[128, TC], [1, 128]]))
                ncum_row = sm.tile([TC, 128], F32, tag="ncum_row", name=f"ncum_row_{b}")
                nc.vector.tensor_tensor(ncum_row[:], attn_row[:], pcr[:], Add)
                nc.gpsimd.dma_start(AP(ncum_d, b * T, [[128, TC], [1, 128]]), ncum_row[:])

            # software pipeline: A(b+1) is emitted before B(b) so the PE's
            # static instruction order never head-of-line blocks on batch b's
            # softmax dependencies
            stage_a(0)
            for b in range(n_batches):
                if b + 1 < n_batches:
                    stage_a(b + 1)
                stage_b(b)

    nc.compile()
    return nc


def make_in_maps(query, prev_attn_weights, prev_attn_weights_cum, memory,
                 memory_sequence_length, W_query, W_memory, conv_kernel, W_loc, v,
                 n_batches=BL, n_cores=NCORES):
    memory_r = f32r_round(memory)
    query_r = f32r_round(query)
    prev_r = f32r_round(prev_attn_weights)
    cum_r = f32r_round(prev_attn_weights_cum)
    wq_r = np.ascontiguousarray(f32r_round(W_query))
    wm_r = np.ascontiguousarray(f32r_round(W_memory))
    # fold conv kernel into W_loc: M2[(c*31+k), a] = sum_f ck[k,c,f] W_loc[f,a]
    m2 = np.einsum(
        "kcf,fa->cka",
        conv_kernel.astype(np.float64),
        W_loc.astype(np.float64),
    ).reshape(KC, A).astype(np.float32)
    m2_rep = np.ascontiguousarray(np.tile(f32r_round(m2), (1, n_batches)))
    shared = {
        "wq": wq_r,
        "wm": wm_r,
        "m2": m2_rep,
        "v": f32r_round(np.concatenate([v, v], axis=1)),
        "ident": np.eye(128, dtype=np.float32),
        "onesrow": np.ones((1, T), dtype=np.float32),
        "onesm": np.ones((1, 128), dtype=np.float32),
    }
    in_maps = []
    for c in range(n_cores):
        b0 = c * n_batches
        sl = slice(b0, b0 + n_batches)
        ppad = np.zeros((n_batches, TP), dtype=np.float32)
        ppad[:, PAD : PAD + T] = prev_r[sl]
        cpad = np.zeros((n_batches, TP), dtype=np.float32)
        cpad[:, PAD : PAD + T] = cum_r[sl]
        # staircase mask: pen[p,c] = NEG*(u[c] + w[p]*z[c]),  t = c*128 + p
        slen = memory_sequence_length[sl].astype(np.int64)
        cs, ps = slen // 128, slen % 128
        masku = np.zeros((n_batches, TC), dtype=np.float32)
        maskz = np.zeros((n_batches, TC), dtype=np.float32)
        maskw = np.zeros((n_batches, 128), dtype=np.float32)
        for j in range(n_batches):
            masku[j, cs[j] + 1 :] = NEG
            if cs[j] < TC:
                maskz[j, cs[j]] = NEG
                maskw[j, ps[j] :] = 1.0
        masku = np.repeat(masku, 2, axis=1)
        maskz = np.repeat(maskz, 2, axis=1)
        in_maps.append(dict(
            shared,
            mem=np.ascontiguousarray(memory_r[sl]),
            ppad=ppad,
            cpad=cpad,
            pcum=np.ascontiguousarray(prev_attn_weights_cum[sl].astype(np.float32)),
            qt=np.ascontiguousarray(query_r[sl].T),
            masku=f32r_round(masku.reshape(1, -1)),
            maskw=maskw.reshape(1, -1),
            maskz=f32r_round(maskz.reshape(1, -1)),
        ))
    return in_maps


_NC_CACHE = {}


def kernel(query, prev_attn_weights, prev_attn_weights_cum, memory,
           memory_sequence_length, W_query, W_memory, conv_kernel, W_loc, v):
    global LAST_EXEC_NS
    query = np.asarray(query, dtype=np.float32)
    prev_attn_weights = np.asarray(prev_attn_weights, dtype=np.float32)
    prev_attn_weights_cum = np.asarray(prev_attn_weights_cum, dtype=np.float32)
    memory = np.asarray(memory, dtype=np.float32)
    memory_sequence_length = np.asarray(memory_sequence_length)
    W_query = np.asarray(W_query, dtype=np.float32)
    W_memory = np.asarray(W_memory, dtype=np.float32)
    conv_kernel = np.asarray(conv_kernel, dtype=np.float32)
    W_loc = np.asarray(W_loc, dtype=np.float32)
    v = np.asarray(v, dtype=np.float32)

    if "nc" not in _NC_CACHE:
        _NC_CACHE["nc"] = build_program(BL)
    nc = _NC_CACHE["nc"]

    in_maps = make_in_maps(
        query, prev_attn_weights, prev_attn_weights_cum, memory,
        memory_sequence_length, W_query, W_memory, conv_kernel, W_loc, v,
    )
    import os
    trace = bool(os.environ.get("KERNEL_TRACE"))
    out = run_bass_kernel_spmd(nc, in_maps, list(range(NCORES)), trace=trace)
    LAST_EXEC_NS = out.exec_time_ns
    res = out.results
    context = np.concatenate([res[c]["ctx_o"] for c in range(NCORES)], axis=0)
    attn = np.concatenate([res[c]["attn_o"] for c in range(NCORES)], axis=0)
    ncum = np.concatenate([res[c]["ncum_o"] for c in range(NCORES)], axis=0)
    return context, attn, ncum


# revision 38
# speedup vs baseline: 1.1696x; 1.1696x over previous
"""Trainium2 Bass kernel for LocationSensitiveAttention.

Strategy (data-parallel over batch, 8 cores x 16 batches):
  per batch b:
    V   [128, 8, 512]  <- memory[b] natural layout (t on partitions, f32r)
    VT  [128, 4, 1024] <- PE-transposed V (d on partitions) via 32 128x128 transposes
    comb[128(a), 512(t)] psum x2 = W_memory^T @ V^T  +  M2plus^T @ XTplus
        where M2plus = [conv_kernel@W_loc ; pq_b] (pq folded via ones row in XT)
    tanh -> [128(a), 1024(t)] sbuf (ACT)
    energies [128(t), 8(tc)] psum via 8 matmuls with v as rhs
    softmax over 1024 = (partitions x 8 cols), no max-sub (|e| <= ||v||_1 ~ 9)
    context [1, 512] psum = sum_t attn[t] * V[t, :]  (8 accumulating matmuls)
All heavy matmul operands are float32r (fp32 RNE-rounded to 11 mantissa bits,
pre-rounded on host), which streams at 1 cyc/row on the PE (4x faster than fp32).
"""
import sys

sys.path.insert(0, "/opt/trn_rl_repo")
import numpy as np

import concourse.bacc as bacc
import concourse.bass as bass
import concourse.mybir as mybir
import concourse.tile as tile
from concourse.bass_utils import run_bass_kernel_spmd

F32 = mybir.dt.float32
F32R = mybir.dt.float32r
AP = bass.AP

NCORES = 8
B, T, D, Q, A, NF, KW = 128, 1024, 512, 1024, 128, 32, 31
BL = B // NCORES          # 16 batches per core
TC = T // 128             # 8 t-chunks
DC = D // 128             # 4 d-chunks
KC = 2 * KW               # 62 conv taps (c-major: rows 0..30 prev, 31..61 cum)
PAD = (KW - 1) // 2       # 15
TP = T + 2 * PAD          # 1054
NEG = -1.0e38

LAST_EXEC_NS = None


def f32r_round(x):
    """Host equivalent of the device fp32->fp32r cast (RNE to 11 mantissa bits)."""
    u = np.ascontiguousarray(x, dtype=np.float32).view(np.uint32).astype(np.uint64)
    tie = (u >> np.uint64(12)) & np.uint64(1)
    r = (((u + np.uint64(0x7FF) + tie) >> np.uint64(12)) << np.uint64(12)).astype(np.uint32)
    return r.view(np.float32)


def build_program(n_batches=BL):
    nc = bacc.Bacc(None, target_bir_lowering=False)
    d = {}

    def inp(name, shape, dt):
        d[name] = nc.dram_tensor(name, list(shape), dt, kind="ExternalInput")
        return d[name]

    def outp(name, shape, dt):
        d[name] = nc.dram_tensor(name, list(shape), dt, kind="ExternalOutput")
        return d[name]

    mem_d = inp("mem", (n_batches, T, D), F32R)
    ppad_d = inp("ppad", (n_batches, TP), F32R)
    cpad_d = inp("cpad", (n_batches, TP), F32R)
    pcum_d = inp("pcum", (n_batches, T), F32)
    qt_d = inp("qt", (Q, n_batches), F32R)
    wq_d = inp("wq", (Q, A), F32R)
    wm_d = inp("wm", (D, A), F32R)
    m2_d = inp("m2", (KC, n_batches * A), F32R)
    v_d = inp("v", (A, 2), F32R)
    ident_d = inp("ident", (128, 128), F32R)
    onesrow_d = inp("onesrow", (1, T), F32R)
    onesm_d = inp("onesm", (1, 128), F32R)
    # staircase mask decomposition: pen[p,c] = NEG*(u[c] + w[p]*z[c])
    # u/z duplicated x2 to match the duplicated-energy column layout
    masku_d = inp("masku", (1, n_batches * 2 * TC), F32R)
    maskw_d = inp("maskw", (1, n_batches * 128), F32R)
    maskz_d = inp("maskz", (1, n_batches * 2 * TC), F32R)

    ctx_d = outp("ctx_o", (n_batches, D), F32)
    attn_d = outp("attn_o", (n_batches, T), F32)
    ncum_d = outp("ncum_o", (n_batches, T), F32)

    Add = mybir.AluOpType.add
    Tanh = mybir.ActivationFunctionType.Tanh
    Exp = mybir.ActivationFunctionType.Exp

    with tile.TileContext(nc) as tc:
        with (
            tc.tile_pool(name="static", bufs=1) as st,
            tc.tile_pool(name="vpool", bufs=28) as vpool,
            tc.tile_pool(name="vtpool", bufs=2) as vtpool,
            tc.tile_pool(name="xtpool", bufs=2) as xtpool,
            tc.tile_pool(name="thpool", bufs=2) as thpool,
            tc.tile_pool(name="small", bufs=4) as sm,
            tc.tile_pool(name="vtps", bufs=3, space=bass.MemorySpace.PSUM) as vtps_p,
            tc.tile_pool(name="comb", bufs=2, space=bass.MemorySpace.PSUM) as comb_p,
            tc.tile_pool(name="ep", bufs=3, space=bass.MemorySpace.PSUM) as ep_p,
        ):
            # ---- statics ----
            wm = st.tile([128, DC, A], F32R, tag="wm")
            nc.sync.dma_start(wm[:], AP(wm_d, 0, [[A, 128], [128 * A, DC], [1, A]]))
            wq = st.tile([128, TC, A], F32R, tag="wq")
            nc.sync.dma_start(wq[:], AP(wq_d, 0, [[A, 128], [128 * A, TC], [1, A]]))
            qt = st.tile([128, TC, n_batches], F32R, tag="qt")
            nc.sync.dma_start(
                qt[:], AP(qt_d, 0, [[n_batches, 128], [128 * n_batches, TC], [1, n_batches]])
            )
            m2p = st.tile([KC + 1, n_batches * A], F32R, tag="m2p")
            nc.sync.dma_start(m2p[0:KC, :], m2_d[:])
            ident = st.tile([128, 128], F32R, tag="ident")
            nc.sync.dma_start(ident[:], ident_d[:])
            vv = st.tile([A, 2], F32R, tag="vv")
            nc.sync.dma_start(vv[:], v_d[:])
            onesm = st.tile([1, 128], F32R, tag="onesm")
            nc.sync.dma_start(onesm[:], onesm_d[:])
            masku = st.tile([1, n_batches * 2 * TC], F32R, tag="masku")
            nc.sync.dma_start(masku[:], masku_d[:])
            maskw = st.tile([1, n_batches * 128], F32R, tag="maskw")
            nc.sync.dma_start(maskw[:], maskw_d[:])
            maskz = st.tile([1, n_batches * 2 * TC], F32R, tag="maskz")
            nc.sync.dma_start(maskz[:], maskz_d[:])

            # ---- pq = query @ W_query, scattered into row 62 of m2p ----
            pq_ps = ep_p.tile([n_batches, A], F32, tag="ep")
            for c in range(TC):
                nc.tensor.matmul(
                    pq_ps[:], qt[:, c, :], wq[:, c, :], start=(c == 0), stop=(c == TC - 1)
                )
            pq_sb = sm.tile([n_batches, A], F32R, tag="pq_sb")
            nc.vector.tensor_copy(pq_sb[:], pq_ps[:])
            nc.sync.dma_start(m2p[KC : KC + 1, :], pq_sb[:])

            state = {}

            def stage_a(b):
                # ---- load memory[b] natural: [128(t%128), 8(tc), 512(d)] ----
                V = [
                    vpool.tile([128, D], F32R, tag="V", name=f"V_{b}_{i}")
                    for i in range(TC)
                ]
                for tcc in range(TC):
                    nc.sync.dma_start(
                        V[tcc][:],
                        AP(mem_d, (b * T + tcc * 128) * D, [[D, 128], [1, D]]),
                    )
                # ---- im2col windows for the location conv (+ ones row) ----
                XT = xtpool.tile([KC + 1, T], F32R, tag="XT", name=f"XT_{b}")
                nc.sync.dma_start(XT[0:KW, :], AP(ppad_d, b * TP, [[1, KW], [1, T]]))
                nc.sync.dma_start(XT[KW:KC, :], AP(cpad_d, b * TP, [[1, KW], [1, T]]))
                nc.sync.dma_start(XT[KC : KC + 1, :], onesrow_d[:])

                # ---- transpose V -> VT [128(d%128), 4(dc), 1024(t)] ----
                VT = vtpool.tile([128, DC, T], F32R, tag="VT", name=f"VT_{b}")
                for h in range(2):
                    for dc in range(DC):
                        ps = vtps_p.tile([128, 512], F32R, tag="vtps")
                        for i in range(4):
                            tcc = h * 4 + i
                            nc.tensor.matmul(
                                ps[:, i * 128 : (i + 1) * 128],
                                V[tcc][:, dc * 128 : (dc + 1) * 128],
                                ident[:],
                                is_transpose=True,
                                start=(i == 0),
                                stop=(i == 3),
                            )
                        eng = nc.vector if (dc % 2 == 0) else nc.scalar
                        if eng is nc.vector:
                            nc.vector.tensor_copy(
                                VT[:, dc, h * 512 : (h + 1) * 512], ps[:]
                            )
                        else:
                            nc.scalar.copy(VT[:, dc, h * 512 : (h + 1) * 512], ps[:])

                # ---- comb = keys^T + ploc^T + pq  (PSUM [128(a), 512(t)] x2) ----
                th = thpool.tile([128, T], F32R, tag="th", name=f"th_{b}")
                for h in range(2):
                    cps = comb_p.tile([128, 512], F32, tag="comb")
                    for dc in range(DC):
                        nc.tensor.matmul(
                            cps[:],
                            wm[:, dc, :],
                            VT[:, dc, h * 512 : (h + 1) * 512],
                            start=(dc == 0),
                            stop=False,
                        )
                    nc.tensor.matmul(
                        cps[:],
                        m2p[:, b * A : (b + 1) * A],
                        XT[:, h * 512 : (h + 1) * 512],
                        start=False,
                        stop=True,
                    )
                    nc.scalar.activation(th[:, h * 512 : (h + 1) * 512], cps[:], Tanh)

                # ---- energies [128(t), 2*8] (each energy duplicated; f32r N>=2) ----
                eps = ep_p.tile([128, 2 * TC], F32, tag="ep", name=f"eps_{b}")
                for tcc in range(TC):
                    nc.tensor.matmul(
                        eps[:, 2 * tcc : 2 * tcc + 2],
                        th[:, tcc * 128 : (tcc + 1) * 128],
                        vv[:],
                        start=(tcc == 0),
                        stop=False,
                    )
                # pen[p,c] = NEG*(u[c] + w[p]*z[c]) as two rank-1 accumulations
                nc.tensor.matmul(
                    eps[:], onesm[:], masku[:, b * 2 * TC : (b + 1) * 2 * TC],
                    start=False, stop=False,
                )
                nc.tensor.matmul(
                    eps[:],
                    maskw[:, b * 128 : (b + 1) * 128],
                    maskz[:, b * 2 * TC : (b + 1) * 2 * TC],
                    start=False, stop=True,
                )

                # ---- softmax over (partition,col), no max-sub needed ----
                # exp in f32r feeds context directly; normalization deferred
                expm = sm.tile([128, TC], F32R, tag="expm", name=f"expm_{b}")
                scol = sm.tile([128, 1], F32, tag="scol", name=f"scol_{b}")
                nc.scalar.activation(
                    expm[:], eps[:, 0 : 2 * TC : 2], Exp, accum_out=scol[:]
                )
                state[b] = (V, expm, scol)

            def stage_b(b):
                V, expm, scol = state.pop(b)
                # ---- context (unnormalized) [1, 512] -- keeps PE path short ----
                cxp = ep_p.tile([1, D], F32, tag="ep", name=f"cxp_{b}")
                for tcc in range(TC):
                    nc.tensor.matmul(
                        cxp[:],
                        expm[:, tcc : tcc + 1],
                        V[tcc][:],
                        start=(tcc == 0),
                        stop=(tcc == TC - 1),
                    )
                # ---- transpose expm -> rows (depends only on exp, not the ladder) ----
                atp = ep_p.tile([TC, 128], F32R, tag="ep", name=f"atp_{b}")
                nc.tensor.matmul(
                    atp[:], expm[:], ident[:],
                    is_transpose=True, start=True, stop=True,
                )
                # ---- normalization path (entirely off the PE stream) ----
                sall = sm.tile([128, 1], F32, tag="sall", name=f"sall_{b}")
                nc.gpsimd.partition_all_reduce(
                    sall[:], scol[:], channels=128, reduce_op=bass.bass_isa.ReduceOp.add
                )
                r128 = sm.tile([128, 1], F32, tag="r128", name=f"r128_{b}")
                nc.vector.reciprocal(r128[:], sall[:])
                cxr = sm.tile([1, D], F32, tag="cxr", name=f"cxr_{b}")
                nc.scalar.mul(cxr[:], cxp[:], r128[0:1, :])
                nc.gpsimd.dma_start(ctx_d[b : b + 1, :], cxr[:])

                exr = sm.tile([TC, 128], F32, tag="exr", name=f"exr_{b}")
                nc.scalar.copy(exr[:], atp[:].bitcast(F32))
                attn_row = sm.tile([TC, 128], F32, tag="attn_row", name=f"attn_row_{b}")
                nc.vector.tensor_scalar_mul(attn_row[:], exr[:], r128[0:TC, :])
                nc.gpsimd.dma_start(AP(attn_d, b * T, [[128, TC], [1, 128]]), attn_row[:])
                # ---- cumulative weights in row layout: (exr * r) + pcum ----
                pcr = sm.tile([TC, 128], F32, tag="pcr", name=f"pcr_{b}")
                nc.gpsimd.dma_start(pcr[:], AP(pcum_d, b * T, [[128, TC], [1, 128]]))
                ncum_row = sm.tile([TC, 128], F32, tag="ncum_row", name=f"ncum_row_{b}")
                nc.vector.scalar_tensor_tensor(
                    ncum_row[:], exr[:], r128[0:TC, :], pcr[:],
                    mybir.AluOpType.mult, Add,
                )
                nc.gpsimd.dma_start(AP(ncum_d, b * T, [[128, TC], [1, 128]]), ncum_row[:])

            # software pipeline: A(b+1) is emitted before B(b) so the PE's
            # static instruction order never head-of-line blocks on batch b's
            # softmax dependencies
            stage_a(0)
            for b in range(n_batches):
                if b + 1 < n_batches:
                    stage_a(b + 1)
                stage_b(b)

    nc.compile()
    return nc


def make_in_maps(query, prev_attn_weights, prev_attn_weights_cum, memory,
                 memory_sequence_length, W_query, W_memory, conv_kernel, W_loc, v,
                 n_batches=BL, n_cores=NCORES):
    memory_r = f32r_round(memory)
    query_r = f32r_round(query)
    prev_r = f32r_round(prev_attn_weights)
    cum_r = f32r_round(prev_attn_weights_cum)
    wq_r = np.ascontiguousarray(f32r_round(W_query))
    wm_r = np.ascontiguousarray(f32r_round(W_memory))
    # fold conv kernel into W_loc: M2[(c*31+k), a] = sum_f ck[k,c,f] W_loc[f,a]
    m2 = np.einsum(
        "kcf,fa->cka",
        conv_kernel.astype(np.float64),
        W_loc.astype(np.float64),
    ).reshape(KC, A).astype(np.float32)
    m2_rep = np.ascontiguousarray(np.tile(f32r_round(m2), (1, n_batches)))
    shared = {
        "wq": wq_r,
        "wm": wm_r,
        "m2": m2_rep,
        "v": f32r_round(np.concatenate([v, v], axis=1)),
        "ident": np.eye(128, dtype=np.float32),
        "onesrow": np.ones((1, T), dtype=np.float32),
        "onesm": np.ones((1, 128), dtype=np.float32),
    }
    in_maps = []
    for c in range(n_cores):
        b0 = c * n_batches
        sl = slice(b0, b0 + n_batches)
        ppad = np.zeros((n_batches, TP), dtype=np.float32)
        ppad[:, PAD : PAD + T] = prev_r[sl]
        cpad = np.zeros((n_batches, TP), dtype=np.float32)
        cpad[:, PAD : PAD + T] = cum_r[sl]
        # staircase mask: pen[p,c] = NEG*(u[c] + w[p]*z[c]),  t = c*128 + p
        slen = memory_sequence_length[sl].astype(np.int64)
        cs, ps = slen // 128, slen % 128
        masku = np.zeros((n_batches, TC), dtype=np.float32)
        maskz = np.zeros((n_batches, TC), dtype=np.float32)
        maskw = np.zeros((n_batches, 128), dtype=np.float32)
        for j in range(n_batches):
            masku[j, cs[j] + 1 :] = NEG
            if cs[j] < TC:
                maskz[j, cs[j]] = NEG
                maskw[j, ps[j] :] = 1.0
        masku = np.repeat(masku, 2, axis=1)
        maskz = np.repeat(maskz, 2, axis=1)
        in_maps.append(dict(
            shared,
            mem=np.ascontiguousarray(memory_r[sl]),
            ppad=ppad,
            cpad=cpad,
            pcum=np.ascontiguousarray(prev_attn_weights_cum[sl].astype(np.float32)),
            qt=np.ascontiguousarray(query_r[sl].T),
            masku=f32r_round(masku.reshape(1, -1)),
            maskw=maskw.reshape(1, -1),
            maskz=f32r_round(maskz.reshape(1, -1)),
        ))
    return in_maps


_NC_CACHE = {}


def kernel(query, prev_attn_weights, prev_attn_weights_cum, memory,
           memory_sequence_length, W_query, W_memory, conv_kernel, W_loc, v):
    global LAST_EXEC_NS
    query = np.asarray(query, dtype=np.float32)
    prev_attn_weights = np.asarray(prev_attn_weights, dtype=np.float32)
    prev_attn_weights_cum = np.asarray(prev_attn_weights_cum, dtype=np.float32)
    memory = np.asarray(memory, dtype=np.float32)
    memory_sequence_length = np.asarray(memory_sequence_length)
    W_query = np.asarray(W_query, dtype=np.float32)
    W_memory = np.asarray(W_memory, dtype=np.float32)
    conv_kernel = np.asarray(conv_kernel, dtype=np.float32)
    W_loc = np.asarray(W_loc, dtype=np.float32)
    v = np.asarray(v, dtype=np.float32)

    if "nc" not in _NC_CACHE:
        _NC_CACHE["nc"] = build_program(BL)
    nc = _NC_CACHE["nc"]

    in_maps = make_in_maps(
        query, prev_attn_weights, prev_attn_weights_cum, memory,
        memory_sequence_length, W_query, W_memory, conv_kernel, W_loc, v,
    )
    import os
    trace = bool(os.environ.get("KERNEL_TRACE"))
    out = run_bass_kernel_spmd(nc, in_maps, list(range(NCORES)), trace=trace)
    LAST_EXEC_NS = out.exec_time_ns
    res = out.results
    context = np.concatenate([res[c]["ctx_o"] for c in range(NCORES)], axis=0)
    attn = np.concatenate([res[c]["attn_o"] for c in range(NCORES)], axis=0)
    ncum = np.concatenate([res[c]["ncum_o"] for c in range(NCORES)], axis=0)
    return context, attn, ncum


# revision 45
# speedup vs baseline: 1.1803x; 1.0092x over previous
"""Trainium2 Bass kernel for LocationSensitiveAttention.

Strategy (data-parallel over batch, 8 cores x 16 batches):
  per batch b:
    V   [128, 8, 512]  <- memory[b] natural layout (t on partitions, f32r)
    VT  [128, 4, 1024] <- PE-transposed V (d on partitions) via 32 128x128 transposes
    comb[128(a), 512(t)] psum x2 = W_memory^T @ V^T  +  M2plus^T @ XTplus
        where M2plus = [conv_kernel@W_loc ; pq_b] (pq folded via ones row in XT)
    tanh -> [128(a), 1024(t)] sbuf (ACT)
    energies [128(t), 8(tc)] psum via 8 matmuls with v as rhs
    softmax over 1024 = (partitions x 8 cols), no max-sub (|e| <= ||v||_1 ~ 9)
    context [1, 512] psum = sum_t attn[t] * V[t, :]  (8 accumulating matmuls)
All heavy matmul operands are float32r (fp32 RNE-rounded to 11 mantissa bits,
pre-rounded on host), which streams at 1 cyc/row on the PE (4x faster than fp32).
"""
import sys

sys.path.insert(0, "/opt/trn_rl_repo")
import numpy as np

import concourse.bacc as bacc
import concourse.bass as bass
import concourse.mybir as mybir
import concourse.tile as tile
from concourse.bass_utils import run_bass_kernel_spmd

F32 = mybir.dt.float32
F32R = mybir.dt.float32r
AP = bass.AP

NCORES = 8
B, T, D, Q, A, NF, KW = 128, 1024, 512, 1024, 128, 32, 31
BL = B // NCORES          # 16 batches per core
TC = T // 128             # 8 t-chunks
DC = D // 128             # 4 d-chunks
KC = 2 * KW               # 62 conv taps (c-major: rows 0..30 prev, 31..61 cum)
PAD = (KW - 1) // 2       # 15
TP = T + 2 * PAD          # 1054
NEG = -1.0e38

LAST_EXEC_NS = None


def f32r_round(x):
    """Host equivalent of the device fp32->fp32r cast (RNE to 11 mantissa bits)."""
    u = np.ascontiguousarray(x, dtype=np.float32).view(np.uint32).astype(np.uint64)
    tie = (u >> np.uint64(12)) & np.uint64(1)
    r = (((u + np.uint64(0x7FF) + tie) >> np.uint64(12)) << np.uint64(12)).astype(np.uint32)
    return r.view(np.float32)


def build_program(n_batches=BL):
    nc = bacc.Bacc(None, target_bir_lowering=False)
    d = {}

    def inp(name, shape, dt):
        d[name] = nc.dram_tensor(name, list(shape), dt, kind="ExternalInput")
        return d[name]

    def outp(name, shape, dt):
        d[name] = nc.dram_tensor(name, list(shape), dt, kind="ExternalOutput")
        return d[name]

    mem_d = inp("mem", (n_batches, T, D), F32R)
    ppad_d = inp("ppad", (n_batches, TP), F32R)
    cpad_d = inp("cpad", (n_batches, TP), F32R)
    pcum_d = inp("pcum", (n_batches, T), F32)
    qt_d = inp("qt", (Q, n_batches), F32R)
    wq_d = inp("wq", (Q, A), F32R)
    wm_d = inp("wm", (D, A), F32R)
    m2_d = inp("m2", (KC, n_batches * A), F32R)
    v_d = inp("v", (A, 2), F32R)
    ident_d = inp("ident", (128, 128), F32R)
    onesrow_d = inp("onesrow", (1, T), F32R)
    onesm_d = inp("onesm", (1, 128), F32R)
    # staircase mask decomposition: pen[p,c] = NEG*(u[c] + w[p]*z[c]),
    # stacked as one K=2 matmul: lhsT rows [ones; w_b], rhs rows [u_b; z_b]
    # (u/z duplicated x2 to match the duplicated-energy column layout)
    maskl_d = inp("maskl", (2, n_batches * 128), F32R)
    maskr_d = inp("maskr", (2, n_batches * 2 * TC), F32R)

    ctx_d = outp("ctx_o", (n_batches, D), F32)
    attn_d = outp("attn_o", (n_batches, T), F32)
    ncum_d = outp("ncum_o", (n_batches, T), F32)

    Add = mybir.AluOpType.add
    Tanh = mybir.ActivationFunctionType.Tanh
    Exp = mybir.ActivationFunctionType.Exp

    with tile.TileContext(nc) as tc:
        with (
            tc.tile_pool(name="static", bufs=1) as st,
            tc.tile_pool(name="vpool", bufs=32) as vpool,
            tc.tile_pool(name="vtpool", bufs=2) as vtpool,
            tc.tile_pool(name="xtpool", bufs=3) as xtpool,
            tc.tile_pool(name="thpool", bufs=2) as thpool,
            tc.tile_pool(name="small", bufs=6) as sm,
            tc.tile_pool(name="vtps", bufs=3, space=bass.MemorySpace.PSUM) as vtps_p,
            tc.tile_pool(name="comb", bufs=2, space=bass.MemorySpace.PSUM) as comb_p,
            tc.tile_pool(name="ep", bufs=3, space=bass.MemorySpace.PSUM) as ep_p,
        ):
            # ---- statics ----
            wm = st.tile([128, DC, A], F32R, tag="wm")
            nc.sync.dma_start(wm[:], AP(wm_d, 0, [[A, 128], [128 * A, DC], [1, A]]))
            wq = st.tile([128, TC, A], F32R, tag="wq")
            nc.sync.dma_start(wq[:], AP(wq_d, 0, [[A, 128], [128 * A, TC], [1, A]]))
            qt = st.tile([128, TC, n_batches], F32R, tag="qt")
            nc.sync.dma_start(
                qt[:], AP(qt_d, 0, [[n_batches, 128], [128 * n_batches, TC], [1, n_batches]])
            )
            m2p = st.tile([KC + 1, n_batches * A], F32R, tag="m2p")
            nc.sync.dma_start(m2p[0:KC, :], m2_d[:])
            ident = st.tile([128, 128], F32R, tag="ident")
            nc.sync.dma_start(ident[:], ident_d[:])
            vv = st.tile([A, 2], F32R, tag="vv")
            nc.sync.dma_start(vv[:], v_d[:])
            maskl = st.tile([2, n_batches * 128], F32R, tag="maskl")
            nc.sync.dma_start(maskl[:], maskl_d[:])
            maskr = st.tile([2, n_batches * 2 * TC], F32R, tag="maskr")
            nc.sync.dma_start(maskr[:], maskr_d[:])

            # PE warmup while the first memory tiles stream in: keeps HAM at
            # full clock and overlaps the initial DMA latency
            for w in range(24):
                wps = vtps_p.tile([128, 128], F32R, tag="vtps", name=f"warm_{w}")
                nc.tensor.matmul(
                    wps[:], ident[:], ident[:],
                    is_transpose=True, start=True, stop=True,
                )

            # ---- pq = query @ W_query, scattered into row 62 of m2p ----
            pq_ps = ep_p.tile([n_batches, A], F32, tag="ep")
            for c in range(TC):
                nc.tensor.matmul(
                    pq_ps[:], qt[:, c, :], wq[:, c, :], start=(c == 0), stop=(c == TC - 1)
                )
            pq_sb = sm.tile([n_batches, A], F32R, tag="pq_sb")
            nc.vector.tensor_copy(pq_sb[:], pq_ps[:])
            nc.sync.dma_start(m2p[KC : KC + 1, :], pq_sb[:])

            state = {}

            def stage_a(b):
                # ---- load memory[b] natural: [128(t%128), 8(tc), 512(d)] ----
                V = [
                    vpool.tile([128, D], F32R, tag="V", name=f"V_{b}_{i}")
                    for i in range(TC)
                ]
                for tcc in range(TC):
                    nc.sync.dma_start(
                        V[tcc][:],
                        AP(mem_d, (b * T + tcc * 128) * D, [[D, 128], [1, D]]),
                    )
                # ---- im2col windows for the location conv (+ ones row) ----
                XT = xtpool.tile([KC + 1, T], F32R, tag="XT", name=f"XT_{b}")
                nc.sync.dma_start(XT[0:KW, :], AP(ppad_d, b * TP, [[1, KW], [1, T]]))
                nc.sync.dma_start(XT[KW:KC, :], AP(cpad_d, b * TP, [[1, KW], [1, T]]))
                nc.sync.dma_start(XT[KC : KC + 1, :], onesrow_d[:])

                # ---- transpose V -> VT [128(d%128), 4(dc), 1024(t)] ----
                VT = vtpool.tile([128, DC, T], F32R, tag="VT", name=f"VT_{b}")
                for h in range(2):
                    for dc in range(DC):
                        ps = vtps_p.tile([128, 512], F32R, tag="vtps")
                        for i in range(4):
                            tcc = h * 4 + i
                            nc.tensor.matmul(
                                ps[:, i * 128 : (i + 1) * 128],
                                V[tcc][:, dc * 128 : (dc + 1) * 128],
                                ident[:],
                                is_transpose=True,
                                start=(i == 0),
                                stop=(i == 3),
                            )
                        eng = nc.vector if (dc % 2 == 0) else nc.scalar
                        if eng is nc.vector:
                            nc.vector.tensor_copy(
                                VT[:, dc, h * 512 : (h + 1) * 512], ps[:]
                            )
                        else:
                            nc.scalar.copy(VT[:, dc, h * 512 : (h + 1) * 512], ps[:])

                # ---- comb = keys^T + ploc^T + pq  (PSUM [128(a), 512(t)] x2) ----
                th = thpool.tile([128, T], F32R, tag="th", name=f"th_{b}")
                for h in range(2):
                    cps = comb_p.tile([128, 512], F32, tag="comb")
                    for dc in range(DC):
                        nc.tensor.matmul(
                            cps[:],
                            wm[:, dc, :],
                            VT[:, dc, h * 512 : (h + 1) * 512],
                            start=(dc == 0),
                            stop=False,
                        )
                    nc.tensor.matmul(
                        cps[:],
                        m2p[:, b * A : (b + 1) * A],
                        XT[:, h * 512 : (h + 1) * 512],
                        start=False,
                        stop=True,
                    )
                    nc.scalar.activation(th[:, h * 512 : (h + 1) * 512], cps[:], Tanh)

                # ---- energies [128(t), 2*8] (each energy duplicated; f32r N>=2) ----
                eps = ep_p.tile([128, 2 * TC], F32, tag="ep", name=f"eps_{b}")
                for tcc in range(TC):
                    nc.tensor.matmul(
                        eps[:, 2 * tcc : 2 * tcc + 2],
                        th[:, tcc * 128 : (tcc + 1) * 128],
                        vv[:],
                        start=(tcc == 0),
                        stop=False,
                    )
                # pen[p,c] = NEG*(u[c] + w[p]*z[c]) as one K=2 accumulation
                nc.tensor.matmul(
                    eps[:],
                    maskl[:, b * 128 : (b + 1) * 128],
                    maskr[:, b * 2 * TC : (b + 1) * 2 * TC],
                    start=False, stop=True,
                )

                # ---- softmax over (partition,col), no max-sub needed ----
                # exp in f32r feeds context directly; normalization deferred
                expm = sm.tile([128, TC], F32R, tag="expm", name=f"expm_{b}")
                scol = sm.tile([128, 1], F32, tag="scol", name=f"scol_{b}")
                nc.scalar.activation(
                    expm[:], eps[:, 0 : 2 * TC : 2], Exp, accum_out=scol[:]
                )
                state[b] = (V, expm, scol)

            def stage_b(b):
                V, expm, scol = state.pop(b)
                # ---- context (unnormalized) [1, 512] -- keeps PE path short ----
                cxp = ep_p.tile([1, D], F32, tag="ep", name=f"cxp_{b}")
                for tcc in range(TC):
                    nc.tensor.matmul(
                        cxp[:],
                        expm[:, tcc : tcc + 1],
                        V[tcc][:],
                        start=(tcc == 0),
                        stop=(tcc == TC - 1),
                    )
                # ---- transpose expm -> rows (depends only on exp, not the ladder) ----
                atp = ep_p.tile([TC, 128], F32R, tag="ep", name=f"atp_{b}")
                nc.tensor.matmul(
                    atp[:], expm[:], ident[:],
                    is_transpose=True, start=True, stop=True,
                )
                # ---- normalization path (entirely off the PE stream) ----
                sall = sm.tile([128, 1], F32, tag="sall", name=f"sall_{b}")
                nc.gpsimd.partition_all_reduce(
                    sall[:], scol[:], channels=128, reduce_op=bass.bass_isa.ReduceOp.add
                )
                r128 = sm.tile([128, 1], F32, tag="r128", name=f"r128_{b}")
                nc.vector.reciprocal(r128[:], sall[:])
                cxr = sm.tile([1, D], F32, tag="cxr", name=f"cxr_{b}")
                nc.scalar.mul(cxr[:], cxp[:], r128[0:1, :])
                nc.gpsimd.dma_start(ctx_d[b : b + 1, :], cxr[:])

                exr = sm.tile([TC, 128], F32, tag="exr", name=f"exr_{b}")
                nc.scalar.copy(exr[:], atp[:].bitcast(F32))
                attn_row = sm.tile([TC, 128], F32, tag="attn_row", name=f"attn_row_{b}")
                nc.vector.tensor_scalar_mul(attn_row[:], exr[:], r128[0:TC, :])
                nc.gpsimd.dma_start(AP(attn_d, b * T, [[128, TC], [1, 128]]), attn_row[:])
                # ---- cumulative weights in row layout: (exr * r) + pcum ----
                pcr = sm.tile([TC, 128], F32, tag="pcr", name=f"pcr_{b}")
                nc.gpsimd.dma_start(pcr[:], AP(pcum_d, b * T, [[128, TC], [1, 128]]))
                ncum_row = sm.tile([TC, 128], F32, tag="ncum_row", name=f"ncum_row_{b}")
                nc.vector.scalar_tensor_tensor(
                    ncum_row[:], exr[:], r128[0:TC, :], pcr[:],
                    mybir.AluOpType.mult, Add,
                )
                nc.gpsimd.dma_start(AP(ncum_d, b * T, [[128, TC], [1, 128]]), ncum_row[:])

            # software pipeline: A(b+1) is emitted before B(b) so the PE's
            # static instruction order never head-of-line blocks on batch b's
            # softmax dependencies
            stage_a(0)
            for b in range(n_batches):
                if b + 1 < n_batches:
                    stage_a(b + 1)
                stage_b(b)

    nc.compile()
    return nc


def make_in_maps(query, prev_attn_weights, prev_attn_weights_cum, memory,
                 memory_sequence_length, W_query, W_memory, conv_kernel, W_loc, v,
                 n_batches=BL, n_cores=NCORES):
    memory_r = f32r_round(memory)
    query_r = f32r_round(query)
    prev_r = f32r_round(prev_attn_weights)
    cum_r = f32r_round(prev_attn_weights_cum)
    wq_r = np.ascontiguousarray(f32r_round(W_query))
    wm_r = np.ascontiguousarray(f32r_round(W_memory))
    # fold conv kernel into W_loc: M2[(c*31+k), a] = sum_f ck[k,c,f] W_loc[f,a]
    m2 = np.einsum(
        "kcf,fa->cka",
        conv_kernel.astype(np.float64),
        W_loc.astype(np.float64),
    ).reshape(KC, A).astype(np.float32)
    m2_rep = np.ascontiguousarray(np.tile(f32r_round(m2), (1, n_batches)))
    shared = {
        "wq": wq_r,
        "wm": wm_r,
        "m2": m2_rep,
        "v": f32r_round(np.concatenate([v, v], axis=1)),
        "ident": np.eye(128, dtype=np.float32),
        "onesrow": np.ones((1, T), dtype=np.float32),
        "onesm": np.ones((1, 128), dtype=np.float32),
    }
    in_maps = []
    for c in range(n_cores):
        b0 = c * n_batches
        sl = slice(b0, b0 + n_batches)
        ppad = np.zeros((n_batches, TP), dtype=np.float32)
        ppad[:, PAD : PAD + T] = prev_r[sl]
        cpad = np.zeros((n_batches, TP), dtype=np.float32)
        cpad[:, PAD : PAD + T] = cum_r[sl]
        # staircase mask: pen[p,c] = NEG*(u[c] + w[p]*z[c]),  t = c*128 + p
        slen = memory_sequence_length[sl].astype(np.int64)
        cs, ps = slen // 128, slen % 128
        masku = np.zeros((n_batches, TC), dtype=np.float32)
        maskz = np.zeros((n_batches, TC), dtype=np.float32)
        maskw = np.zeros((n_batches, 128), dtype=np.float32)
        for j in range(n_batches):
            masku[j, cs[j] + 1 :] = NEG
            if cs[j] < TC:
                maskz[j, cs[j]] = NEG
                maskw[j, ps[j] :] = 1.0
        masku = np.repeat(masku, 2, axis=1)
        maskz = np.repeat(maskz, 2, axis=1)
        maskl = np.stack([np.ones((n_batches, 128), np.float32), maskw], axis=0)
        maskr = np.stack([masku, maskz], axis=0)
        in_maps.append(dict(
            shared,
            mem=np.ascontiguousarray(memory_r[sl]),
            ppad=ppad,
            cpad=cpad,
            pcum=np.ascontiguousarray(prev_attn_weights_cum[sl].astype(np.float32)),
            qt=np.ascontiguousarray(query_r[sl].T),
            maskl=maskl.reshape(2, -1),
            maskr=f32r_round(maskr.reshape(2, -1)),
        ))
    return in_maps


_NC_CACHE = {}


def kernel(query, prev_attn_weights, prev_attn_weights_cum, memory,
           memory_sequence_length, W_query, W_memory, conv_kernel, W_loc, v):
    global LAST_EXEC_NS
    query = np.asarray(query, dtype=np.float32)
    prev_attn_weights = np.asarray(prev_attn_weights, dtype=np.float32)
    prev_attn_weights_cum = np.asarray(prev_attn_weights_cum, dtype=np.float32)
    memory = np.asarray(memory, dtype=np.float32)
    memory_sequence_length = np.asarray(memory_sequence_length)
    W_query = np.asarray(W_query, dtype=np.float32)
    W_memory = np.asarray(W_memory, dtype=np.float32)
    conv_kernel = np.asarray(conv_kernel, dtype=np.float32)
    W_loc = np.asarray(W_loc, dtype=np.float32)
    v = np.asarray(v, dtype=np.float32)

    if "nc" not in _NC_CACHE:
        _NC_CACHE["nc"] = build_program(BL)
    nc = _NC_CACHE["nc"]

    in_maps = make_in_maps(
        query, prev_attn_weights, prev_attn_weights_cum, memory,
        memory_sequence_length, W_query, W_memory, conv_kernel, W_loc, v,
    )
    import os
    trace = bool(os.environ.get("KERNEL_TRACE"))
    out = run_bass_kernel_spmd(nc, in_maps, list(range(NCORES)), trace=trace)
    LAST_EXEC_NS = out.exec_time_ns
    res = out.results
    context = np.concatenate([res[c]["ctx_o"] for c in range(NCORES)], axis=0)
    attn = np.concatenate([res[c]["attn_o"] for c in range(NCORES)], axis=0)
    ncum = np.concatenate([res[c]["ncum_o"] for c in range(NCORES)], axis=0)
    return context, attn, ncum


# revision 53
# speedup vs baseline: 1.7967x; 1.5221x over previous
"""Trainium2 Bass kernel for LocationSensitiveAttention.

Strategy (data-parallel over batch, 8 cores x 16 batches):
  per batch b:
    V   [128, 8, 512]  <- memory[b] natural layout (t on partitions, f32r)
    VT  [128, 4, 1024] <- PE-transposed V (d on partitions) via 32 128x128 transposes
    comb[128(a), 512(t)] psum x2 = W_memory^T @ V^T  +  M2plus^T @ XTplus
        where M2plus = [conv_kernel@W_loc ; pq_b] (pq folded via ones row in XT)
    tanh -> [128(a), 1024(t)] sbuf (ACT)
    energies [128(t), 8(tc)] psum via 8 matmuls with v as rhs
    softmax over 1024 = (partitions x 8 cols), no max-sub (|e| <= ||v||_1 ~ 9)
    context [1, 512] psum = sum_t attn[t] * V[t, :]  (8 accumulating matmuls)
All heavy matmul operands are float32r (fp32 RNE-rounded to 11 mantissa bits,
pre-rounded on host), which streams at 1 cyc/row on the PE (4x faster than fp32).
"""
import sys

sys.path.insert(0, "/opt/trn_rl_repo")
import numpy as np

import concourse.bacc as bacc
import concourse.bass as bass
import concourse.mybir as mybir
import concourse.tile as tile
from concourse.bass_utils import run_bass_kernel_spmd

F32 = mybir.dt.float32
F32R = mybir.dt.float32r
AP = bass.AP

NCORES = 8
B, T, D, Q, A, NF, KW = 128, 1024, 512, 1024, 128, 32, 31
BL = B // NCORES          # 16 batches per core
TC = T // 128             # 8 t-chunks
DC = D // 128             # 4 d-chunks
KC = 2 * KW               # 62 conv taps (c-major: rows 0..30 prev, 31..61 cum)
PAD = (KW - 1) // 2       # 15
TP = T + 2 * PAD          # 1054
NEG = -1.0e38

LAST_EXEC_NS = None


def f32r_round(x):
    """Host equivalent of the device fp32->fp32r cast (RNE to 11 mantissa bits)."""
    u = np.ascontiguousarray(x, dtype=np.float32).view(np.uint32).astype(np.uint64)
    tie = (u >> np.uint64(12)) & np.uint64(1)
    r = (((u + np.uint64(0x7FF) + tie) >> np.uint64(12)) << np.uint64(12)).astype(np.uint32)
    return r.view(np.float32)


def build_program(n_batches=BL):
    nc = bacc.Bacc(None, target_bir_lowering=False)
    d = {}

    def inp(name, shape, dt):
        d[name] = nc.dram_tensor(name, list(shape), dt, kind="ExternalInput")
        return d[name]

    def outp(name, shape, dt):
        d[name] = nc.dram_tensor(name, list(shape), dt, kind="ExternalOutput")
        return d[name]

    mem_d = inp("mem", (n_batches, T, D), F32R)
    pcpad_d = inp("pcpad", (n_batches, 2, TP), F32R)
    pcum_d = inp("pcum", (n_batches, T), F32)
    qt_d = inp("qt", (Q, n_batches), F32R)
    wq_d = inp("wq", (Q, A), F32R)
    wm_d = inp("wm", (D, A), F32R)
    m2_d = inp("m2", (KC, n_batches * A), F32R)
    v_d = inp("v", (A, 2), F32R)
    ident_d = inp("ident", (128, 128), F32R)
    onesrow_d = inp("onesrow", (1, T), F32R)
    onesm_d = inp("onesm", (1, 128), F32R)
    # staircase mask decomposition: pen[p,c] = NEG*(u[c] + w[p]*z[c]),
    # stacked as one K=2 matmul: lhsT rows [ones; w_b], rhs rows [u_b; z_b]
    # (u/z duplicated x2 to match the duplicated-energy column layout)
    maskl_d = inp("maskl", (2, n_batches * 128), F32R)
    maskr_d = inp("maskr", (2, n_batches * 2 * TC), F32R)

    ctx_d = outp("ctx_o", (n_batches, D), F32)
    attn_d = outp("attn_o", (n_batches, T), F32)
    ncum_d = outp("ncum_o", (n_batches, T), F32)

    Add = mybir.AluOpType.add
    Tanh = mybir.ActivationFunctionType.Tanh
    Exp = mybir.ActivationFunctionType.Exp

    with tile.TileContext(nc) as tc:
        with (
            tc.tile_pool(name="static", bufs=1) as st,
            tc.tile_pool(name="vpool", bufs=4) as vpool,
            tc.tile_pool(name="vtpool", bufs=2) as vtpool,
            tc.tile_pool(name="xtpool", bufs=3) as xtpool,
            tc.tile_pool(name="thpool", bufs=2) as thpool,
            tc.tile_pool(name="small", bufs=6) as sm,
            tc.tile_pool(name="vtps", bufs=3, space=bass.MemorySpace.PSUM) as vtps_p,
            tc.tile_pool(name="comb", bufs=2, space=bass.MemorySpace.PSUM) as comb_p,
            tc.tile_pool(name="ep", bufs=3, space=bass.MemorySpace.PSUM) as ep_p,
        ):
            # ---- statics ----
            wm = st.tile([128, DC, A], F32R, tag="wm")
            nc.sync.dma_start(wm[:], AP(wm_d, 0, [[A, 128], [128 * A, DC], [1, A]]))
            wq = st.tile([128, TC, A], F32R, tag="wq")
            nc.sync.dma_start(wq[:], AP(wq_d, 0, [[A, 128], [128 * A, TC], [1, A]]))
            qt = st.tile([128, TC, n_batches], F32R, tag="qt")
            nc.sync.dma_start(
                qt[:], AP(qt_d, 0, [[n_batches, 128], [128 * n_batches, TC], [1, n_batches]])
            )
            m2p = st.tile([KC + 1, n_batches * A], F32R, tag="m2p")
            nc.sync.dma_start(m2p[0:KC, :], m2_d[:])
            ident = st.tile([128, 128], F32R, tag="ident")
            nc.sync.dma_start(ident[:], ident_d[:])
            vv = st.tile([A, 2], F32R, tag="vv")
            nc.sync.dma_start(vv[:], v_d[:])
            maskl = st.tile([2, n_batches * 128], F32R, tag="maskl")
            nc.sync.dma_start(maskl[:], maskl_d[:])
            maskr = st.tile([2, n_batches * 2 * TC], F32R, tag="maskr")
            nc.sync.dma_start(maskr[:], maskr_d[:])

            # PE warmup while the first memory tiles stream in: keeps HAM at
            # full clock and overlaps the initial DMA latency
            for w in range(64):
                wps = vtps_p.tile([128, 128], F32R, tag="vtps", name=f"warm_{w}")
                nc.tensor.matmul(
                    wps[:], ident[:], ident[:],
                    is_transpose=True, start=True, stop=True,
                )

            # ---- pq = query @ W_query, scattered into row 62 of m2p ----
            pq_ps = ep_p.tile([n_batches, A], F32, tag="ep")
            for c in range(TC):
                nc.tensor.matmul(
                    pq_ps[:], qt[:, c, :], wq[:, c, :], start=(c == 0), stop=(c == TC - 1)
                )
            pq_sb = sm.tile([n_batches, A], F32R, tag="pq_sb")
            nc.vector.tensor_copy(pq_sb[:], pq_ps[:])
            nc.sync.dma_start(m2p[KC : KC + 1, :], pq_sb[:])

            state = {}

            def stage_a(b):
                # ---- load memory[b] natural: [128(t%128), 8(tc), 512(d)] ----
                V = vpool.tile([128, TC, D], F32R, tag="V", name=f"V_{b}")
                nc.sync.dma_start(
                    V[:], AP(mem_d, b * T * D, [[D, 128], [128 * D, TC], [1, D]])
                )
                # ---- im2col windows for the location conv (+ ones row) ----
                XT = xtpool.tile([KC + 1, T], F32R, tag="XT", name=f"XT_{b}")
                nc.gpsimd.dma_start(
                    XT[0:KC, :],
                    AP(pcpad_d, b * 2 * TP, [[TP, 2], [1, KW], [1, T]]),
                )
                nc.gpsimd.dma_start(XT[KC : KC + 1, :], onesrow_d[:])

                # ---- transpose V -> VT [128(d%128), 4(dc), 1024(t)] ----
                VT = vtpool.tile([128, DC, T], F32R, tag="VT", name=f"VT_{b}")
                for h in range(2):
                    for dc in range(DC):
                        ps = vtps_p.tile([128, 512], F32R, tag="vtps")
                        for i in range(4):
                            tcc = h * 4 + i
                            nc.tensor.matmul(
                                ps[:, i * 128 : (i + 1) * 128],
                                V[:, tcc, dc * 128 : (dc + 1) * 128],
                                ident[:],
                                is_transpose=True,
                                start=(i == 0),
                                stop=(i == 3),
                            )
                        eng = nc.vector if (dc % 2 == 0) else nc.scalar
                        if eng is nc.vector:
                            nc.vector.tensor_copy(
                                VT[:, dc, h * 512 : (h + 1) * 512], ps[:]
                            )
                        else:
                            nc.scalar.copy(VT[:, dc, h * 512 : (h + 1) * 512], ps[:])

                # ---- comb = keys^T + ploc^T + pq  (PSUM [128(a), 512(t)] x2) ----
                th = thpool.tile([128, T], F32R, tag="th", name=f"th_{b}")
                for h in range(2):
                    cps = comb_p.tile([128, 512], F32, tag="comb")
                    for dc in range(DC):
                        nc.tensor.matmul(
                            cps[:],
                            wm[:, dc, :],
                            VT[:, dc, h * 512 : (h + 1) * 512],
                            start=(dc == 0),
                            stop=False,
                        )
                    nc.tensor.matmul(
                        cps[:],
                        m2p[:, b * A : (b + 1) * A],
                        XT[:, h * 512 : (h + 1) * 512],
                        start=False,
                        stop=True,
                    )
                    nc.scalar.activation(th[:, h * 512 : (h + 1) * 512], cps[:], Tanh)

                # ---- energies [128(t), 2*8] (each energy duplicated; f32r N>=2) ----
                eps = ep_p.tile([128, 2 * TC], F32, tag="ep", name=f"eps_{b}")
                for tcc in range(TC):
                    nc.tensor.matmul(
                        eps[:, 2 * tcc : 2 * tcc + 2],
                        th[:, tcc * 128 : (tcc + 1) * 128],
                        vv[:],
                        start=(tcc == 0),
                        stop=False,
                    )
                # pen[p,c] = NEG*(u[c] + w[p]*z[c]) as one K=2 accumulation
                nc.tensor.matmul(
                    eps[:],
                    maskl[:, b * 128 : (b + 1) * 128],
                    maskr[:, b * 2 * TC : (b + 1) * 2 * TC],
                    start=False, stop=True,
                )

                # ---- softmax over (partition,col), no max-sub needed ----
                # exp in f32r feeds context directly; normalization deferred
                expm = sm.tile([128, TC], F32R, tag="expm", name=f"expm_{b}")
                scol = sm.tile([128, 1], F32, tag="scol", name=f"scol_{b}")
                nc.scalar.activation(
                    expm[:], eps[:, 0 : 2 * TC : 2], Exp, accum_out=scol[:]
                )
                state[b] = (V, expm, scol)

            def stage_b(b):
                V, expm, scol = state.pop(b)
                # ---- context (unnormalized) [1, 512] -- keeps PE path short ----
                cxp = ep_p.tile([1, D], F32, tag="ep", name=f"cxp_{b}")
                for tcc in range(TC):
                    nc.tensor.matmul(
                        cxp[:],
                        expm[:, tcc : tcc + 1],
                        V[:, tcc, :],
                        start=(tcc == 0),
                        stop=(tcc == TC - 1),
                    )
                # ---- transpose expm -> rows (depends only on exp, not the ladder) ----
                atp = ep_p.tile([TC, 128], F32R, tag="ep", name=f"atp_{b}")
                nc.tensor.matmul(
                    atp[:], expm[:], ident[:],
                    is_transpose=True, start=True, stop=True,
                )
                # ---- normalization path (entirely off the PE stream) ----
                sall = sm.tile([128, 1], F32, tag="sall", name=f"sall_{b}")
                nc.gpsimd.partition_all_reduce(
                    sall[:], scol[:], channels=128, reduce_op=bass.bass_isa.ReduceOp.add
                )
                r128 = sm.tile([128, 1], F32, tag="r128", name=f"r128_{b}")
                nc.vector.reciprocal(r128[:], sall[:])
                cxr = sm.tile([1, D], F32, tag="cxr", name=f"cxr_{b}")
                nc.scalar.mul(cxr[:], cxp[:], r128[0:1, :])
                nc.gpsimd.dma_start(ctx_d[b : b + 1, :], cxr[:])

                exr = sm.tile([TC, 128], F32, tag="exr", name=f"exr_{b}")
                nc.scalar.copy(exr[:], atp[:].bitcast(F32))
                attn_row = sm.tile([TC, 128], F32, tag="attn_row", name=f"attn_row_{b}")
                nc.vector.tensor_scalar_mul(attn_row[:], exr[:], r128[0:TC, :])
                nc.gpsimd.dma_start(AP(attn_d, b * T, [[128, TC], [1, 128]]), attn_row[:])
                # ---- cumulative weights in row layout: (exr * r) + pcum ----
                pcr = sm.tile([TC, 128], F32, tag="pcr", name=f"pcr_{b}")
                nc.gpsimd.dma_start(pcr[:], AP(pcum_d, b * T, [[128, TC], [1, 128]]))
                ncum_row = sm.tile([TC, 128], F32, tag="ncum_row", name=f"ncum_row_{b}")
                nc.vector.scalar_tensor_tensor(
                    ncum_row[:], exr[:], r128[0:TC, :], pcr[:],
                    mybir.AluOpType.mult, Add,
                )
                nc.gpsimd.dma_start(AP(ncum_d, b * T, [[128, TC], [1, 128]]), ncum_row[:])

            # software pipeline: A(b+1) is emitted before B(b) so the PE's
            # static instruction order never head-of-line blocks on batch b's
            # softmax dependencies
            stage_a(0)
            for b in range(n_batches):
                if b + 1 < n_batches:
                    stage_a(b + 1)
                stage_b(b)

    nc.compile()
    return nc


def make_in_maps(query, prev_attn_weights, prev_attn_weights_cum, memory,
                 memory_sequence_length, W_query, W_memory, conv_kernel, W_loc, v,
                 n_batches=BL, n_cores=NCORES):
    memory_r = f32r_round(memory)
    query_r = f32r_round(query)
    prev_r = f32r_round(prev_attn_weights)
    cum_r = f32r_round(prev_attn_weights_cum)
    wq_r = np.ascontiguousarray(f32r_round(W_query))
    wm_r = np.ascontiguousarray(f32r_round(W_memory))
    # fold conv kernel into W_loc: M2[(c*31+k), a] = sum_f ck[k,c,f] W_loc[f,a]
    m2 = np.einsum(
        "kcf,fa->cka",
        conv_kernel.astype(np.float64),
        W_loc.astype(np.float64),
    ).reshape(KC, A).astype(np.float32)
    m2_rep = np.ascontiguousarray(np.tile(f32r_round(m2), (1, n_batches)))
    shared = {
        "wq": wq_r,
        "wm": wm_r,
        "m2": m2_rep,
        "v": f32r_round(np.concatenate([v, v], axis=1)),
        "ident": np.eye(128, dtype=np.float32),
        "onesrow": np.ones((1, T), dtype=np.float32),
        "onesm": np.ones((1, 128), dtype=np.float32),
    }
    in_maps = []
    for c in range(n_cores):
        b0 = c * n_batches
        sl = slice(b0, b0 + n_batches)
        pcpad = np.zeros((n_batches, 2, TP), dtype=np.float32)
        pcpad[:, 0, PAD : PAD + T] = prev_r[sl]
        pcpad[:, 1, PAD : PAD + T] = cum_r[sl]
        # staircase mask: pen[p,c] = NEG*(u[c] + w[p]*z[c]),  t = c*128 + p
        slen = memory_sequence_length[sl].astype(np.int64)
        cs, ps = slen // 128, slen % 128
        masku = np.zeros((n_batches, TC), dtype=np.float32)
        maskz = np.zeros((n_batches, TC), dtype=np.float32)
        maskw = np.zeros((n_batches, 128), dtype=np.float32)
        for j in range(n_batches):
            masku[j, cs[j] + 1 :] = NEG
            if cs[j] < TC:
                maskz[j, cs[j]] = NEG
                maskw[j, ps[j] :] = 1.0
        masku = np.repeat(masku, 2, axis=1)
        maskz = np.repeat(maskz, 2, axis=1)
        maskl = np.stack([np.ones((n_batches, 128), np.float32), maskw], axis=0)
        maskr = np.stack([masku, maskz], axis=0)
        in_maps.append(dict(
            shared,
            mem=np.ascontiguousarray(memory_r[sl]),
            pcpad=pcpad,
            pcum=np.ascontiguousarray(prev_attn_weights_cum[sl].astype(np.float32)),
            qt=np.ascontiguousarray(query_r[sl].T),
            maskl=maskl.reshape(2, -1),
            maskr=f32r_round(maskr.reshape(2, -1)),
        ))
    return in_maps


_NC_CACHE = {}


def kernel(query, prev_attn_weights, prev_attn_weights_cum, memory,
           memory_sequence_length, W_query, W_memory, conv_kernel, W_loc, v):
    global LAST_EXEC_NS
    query = np.asarray(query, dtype=np.float32)
    prev_attn_weights = np.asarray(prev_attn_weights, dtype=np.float32)
    prev_attn_weights_cum = np.asarray(prev_attn_weights_cum, dtype=np.float32)
    memory = np.asarray(memory, dtype=np.float32)
    memory_sequence_length = np.asarray(memory_sequence_length)
    W_query = np.asarray(W_query, dtype=np.float32)
    W_memory = np.asarray(W_memory, dtype=np.float32)
    conv_kernel = np.asarray(conv_kernel, dtype=np.float32)
    W_loc = np.asarray(W_loc, dtype=np.float32)
    v = np.asarray(v, dtype=np.float32)

    if "nc" not in _NC_CACHE:
        _NC_CACHE["nc"] = build_program(BL)
    nc = _NC_CACHE["nc"]

    in_maps = make_in_maps(
        query, prev_attn_weights, prev_attn_weights_cum, memory,
        memory_sequence_length, W_query, W_memory, conv_kernel, W_loc, v,
    )
    import os
    trace = bool(os.environ.get("KERNEL_TRACE"))
    out = run_bass_kernel_spmd(nc, in_maps, list(range(NCORES)), trace=trace)
    LAST_EXEC_NS = out.exec_time_ns
    res = out.results
    context = np.concatenate([res[c]["ctx_o"] for c in range(NCORES)], axis=0)
    attn = np.concatenate([res[c]["attn_o"] for c in range(NCORES)], axis=0)
    ncum = np.concatenate([res[c]["ncum_o"] for c in range(NCORES)], axis=0)
    return context, attn, ncum
